# revision 28
# baseline (speedup 1.0000x reference)
"""Trainium2 Bass kernel for causal attention layer (N=4, T=S=4096, D=256, f32).

Sharding: 8 cores = 4 batches x 2-way split of T. Each batch's 32 query
row-blocks (128 rows each) are split by parity: core parity 0 gets even
global blocks, parity 1 odd blocks. Causal boundaries are enforced by two
per-core [128,128] mask tiles supplied as input data, so the instruction
stream is identical on all 8 cores (no collectives).

Algebraic restructuring vs the naive layer (validated to 9e-7 in f32):
  scores[t,s] = (Wq query_t + bq) . (Wk key_s + bk)
              = query_t . M key_s + beta_s + c_t,   M = Wq^T Wk
  c_t is constant over s -> softmax-invariant -> dropped.
  beta_s = (Wk^T bq) . key_s (+ bq.bk, also invariant): folded on the HOST
  into a per-row scale of value: vg_s = exp(beta_s/16) * [value_s | 1], so
  the device uses RAW key (no k-projection) and RAW scaled value (no
  v-projection); softmax denominator comes from the g column of vg.
  y = (attn @ vg[:, :256]) / denom @ Wv^T + bv   (Wv applied at the END to
  the [t,256] normalized output - 2x fewer projection FLOPs than projecting
  k and v, and only q-side work scales with T).

Device algorithm per core (bf16 matmul operands, f32 PSUM accumulation):
  qMT = M^T @ queryT            [d, t]  (PE + DVE copy)
  kT, vg loaded raw by DMA      [d, s], [s, 258]
  per 512-wide t-superblock J, per 128-row s-chunk:
    scoresT[s, t] = kT_chunk.T @ qMT_block          (PE, 2 d-chunk matmuls)
    attnT = exp(scoresT / 16)                       (ScalarE)
    diagonal-boundary block multiplied by a mask tile (DVE)
    U[m] += attnT[:, block m].T @ vg[chunk]         (PE) -> [t=128, 258]
  Tail per slot (3-stage, interleaved with the pair stream so the PE never
  head-of-line blocks on DVE): A: recip+normalize (DVE); B: 2 PE transposes
  + DVE copy; C: 2 Wv matmuls (PE) + bias add (DVE) + DMA out.
"""
import os
import numpy as np

N, T, S, D = 4, 4096, 4096, 256
NCORES = 8
TSH = T // 2          # 2048 query rows per core
NBLK = TSH // 128     # 16 slots per core
NSB = 4               # superblocks of 512 t rows
SCALE = 1.0 / 16.0    # 1/sqrt(D)

# packed constants layout
# wgt (bf16): [M0|M1|WvT0|WvT1|I]  -> 4*256 + 128 cols
# msk (bf16): [mska|mskb]          -> 256 cols
# bvt (f32):  bv broadcast         -> 256 cols
CW = 2 * 256 + 2 * 258 + 258

_CACHE = {}


def _build(loop_r=None):
    from contextlib import ExitStack
    import concourse.bass as bass
    import concourse.tile as tile
    from concourse import bacc, mybir

    f32 = mybir.dt.float32
    bf16 = mybir.dt.bfloat16
    fp8 = mybir.dt.float8e4
    DR = mybir.MatmulPerfMode.DoubleRow
    nc = bacc.Bacc("TRN2", target_bir_lowering=False, debug=False,
                   num_devices=NCORES)

    qT_d = nc.dram_tensor("qT", [D, TSH], bf16, kind="ExternalInput")
    kT_d = nc.dram_tensor("kT", [D, S], bf16, kind="ExternalInput")
    vg_d = nc.dram_tensor("vg", [D, S], bf16, kind="ExternalInput")
    g_d = nc.dram_tensor("g", [1, S], bf16, kind="ExternalInput")
    wg_d = nc.dram_tensor("wgt", [128, CW], bf16, kind="ExternalInput")
    mk_d = nc.dram_tensor("msk", [128, 256], bf16, kind="ExternalInput")
    out_d = nc.dram_tensor("out", [TSH, D], f32, kind="ExternalOutput")

    Exp = mybir.ActivationFunctionType.Exp

    with tile.TileContext(nc) as tc, ExitStack() as _loopctx:
        if loop_r is not None:
            _loopctx.enter_context(tc.For_i(0, loop_r, 1))
        with (
            tc.tile_pool(name="const", bufs=1) as cpool,          # persistent
            tc.tile_pool(name="stage", bufs=3) as spool,          # input staging
            tc.tile_pool(name="work", bufs=8) as wpool,           # attn tiles
            tc.tile_pool(name="small", bufs=4) as smpool,         # tail sbuf
            tc.tile_pool(name="ps2", bufs=4, space="PSUM") as pspool,
            tc.tile_pool(name="psu", bufs=1, space="PSUM") as upool,
        ):
            # ---- persistent SBUF tensors -------------------------------
            wgt = cpool.tile([128, CW], bf16, tag="wgt", name="wgt")
            g_sb = cpool.tile([1, S], bf16, tag="g", name="g")
            msk = cpool.tile([128, 256], bf16, tag="msk", name="msk")
            qMT_sb = [cpool.tile([128, TSH], bf16, tag=f"qMp{i}", name=f"qMp{i}") for i in range(2)]
            kT_sb = [cpool.tile([128, S], bf16, tag=f"kTp{i}", name=f"kTp{i}") for i in range(2)]
            v_sb = [cpool.tile([128, D + 2], bf16, tag=f"v{i}", name=f"v{i}")
                    for i in range(S // 128)]

            nc.sync.dma_start(wgt[:, 0:512], wg_d[:, 0:512])
            nc.scalar.dma_start(msk[:], mk_d[:, :])
            nc.sync.dma_start(wgt[:, 512:CW], wg_d[:, 512:CW])
            nc.scalar.dma_start(g_sb[:], g_d[:, :])

            # tiny exp at t=0: hoists the ACT table load under the DMA wait
            warm = cpool.tile([128, 1], f32, tag="warm", name="warm")
            nc.scalar.activation(warm[:], msk[:, 0:1], Exp, scale=0.0)

            m_sb = [wgt[:, 256 * i:256 * (i + 1)] for i in range(2)]
            wva_sb = [wgt[:, 512 + 258 * i:512 + 258 * (i + 1)] for i in range(2)]
            bvrow = wgt[0:1, 1028:1028 + 258]
            mska = msk[:, 0:128]
            mskb = msk[:, 128:256]

            # ---- q projection: qMT[d, t] = M^T @ queryT ----------------
            def emit_qproj(tb):
                qin = [spool.tile([128, 512], bf16, tag=f"qin{i}", name=f"qin{i}") for i in range(2)]
                for i in range(2):
                    nc.sync.dma_start(qin[i][:],
                                      qT_d[128 * i:128 * (i + 1), 512 * tb:512 * (tb + 1)])
                for o in range(2):
                    ps = pspool.tile([128, 512], f32, tag="ps2", name="ps2")
                    for i in range(2):
                        nc.tensor.matmul(ps[:], m_sb[i][:, 128 * o:128 * (o + 1)],
                                         qin[i][:], start=(i == 0), stop=(i == 1))
                    nc.vector.tensor_copy(qMT_sb[o][:, 512 * tb:512 * (tb + 1)],
                                          ps[:])

            # ---- k load raw; v: project (g*value) through Wv + g-bias --
            def emit_kvload(sb):
                for i in range(2):
                    nc.gpsimd.dma_start(
                        kT_sb[i][:, 512 * sb:512 * (sb + 1)],
                        kT_d[128 * i:128 * (i + 1), 512 * sb:512 * (sb + 1)])
                vin = [spool.tile([128, 512], bf16, tag=f"vin{i}", name=f"vin{i}")
                       for i in range(2)]
                for i in range(2):
                    nc.gpsimd.dma_start(
                        vin[i][:],
                        vg_d[128 * i:128 * (i + 1), 512 * sb:512 * (sb + 1)])
                for si in range(4):
                    c = 4 * sb + si
                    ps = pspool.tile([128, 512], f32, tag="ps2", name="ps2")
                    for i in range(2):
                        nc.tensor.matmul(ps[:, 0:D + 2],
                                         vin[i][:, 128 * si:128 * (si + 1)],
                                         wva_sb[i][:], start=(i == 0),
                                         stop=False)
                    nc.tensor.matmul(ps[:, 0:D + 2],
                                     g_sb[0:1, 128 * c:128 * (c + 1)],
                                     bvrow[:], start=False, stop=True)
                    nc.vector.tensor_copy(v_sb[c][:], ps[:, 0:D + 2])

            pairs = [(J, cp) for J in range(NSB) for cp in range(4 * J + 4)]
            u_ps = {}

            def emit_scores(J, cp):
                c0 = 2 * cp
                at = wpool.tile([128, 1024], bf16, tag="att", name="att")
                dga = dgb = None
                for e in range(2):
                    c = c0 + e
                    o = c - 8 * J
                    off = 0 if o < 0 else 128 * (o // 2)
                    sc = pspool.tile([128, 512], f32, tag="ps2", name="ps2")
                    for i in range(2):
                        nc.tensor.matmul(
                            sc[:, off:512],
                            kT_sb[i][:, 128 * c:128 * (c + 1)],
                            qMT_sb[i][:, 512 * J + off:512 * (J + 1)],
                            start=(i == 0), stop=(i == 1))
                    nc.scalar.activation(at[:, 512 * e + off:512 * (e + 1)],
                                         sc[:, off:512], Exp, scale=SCALE)
                    if o >= 0:
                        kk = o // 2
                        dg = wpool.tile([128, 128], bf16,
                                        tag=("dga" if e == 0 else "dgb"),
                                        name=("dga" if e == 0 else "dgb"))
                        nc.vector.tensor_mul(
                            dg[:], at[:, 512 * e + 128 * kk:512 * e + 128 * (kk + 1)],
                            mska[:] if e == 0 else mskb[:])
                        if e == 0:
                            dga = dg
                        else:
                            dgb = dg
                return at, dga, dgb

            tails = []            # (J, m, stage, state)

            def emit_U(J, cp, tiles):
                at, dga, dgb = tiles
                if cp == 0:
                    u_ps[J] = [upool.tile([128, D + 2], f32, tag=f"u{m}", name=f"u{m}")
                               for m in range(4)]
                for e in range(2):
                    c = 2 * cp + e
                    o = c - 8 * J
                    m_min = 0 if o < 0 else o // 2
                    ms = [m for m in range(m_min, 4) if c <= 8 * J + 2 * m + 1]
                    if o >= 0 and ms and ms[0] == o // 2:
                        ms = ms[1:] + ms[:1]      # diag (mask-gated) block last
                    for m in ms:
                        lastc = 8 * J + 2 * m + 1
                        if o >= 0 and m == o // 2:
                            lhsT = (dga if e == 0 else dgb)[:]
                        else:
                            lhsT = at[:, 512 * e + 128 * m:512 * e + 128 * (m + 1)]
                        nc.tensor.matmul(u_ps[J][m][:], lhsT, v_sb[c][:],
                                         start=(c == 0), stop=(c == lastc),
                                         skip_group_check=True)
                m_done = cp - 4 * J       # slot whose accumulation just closed
                if 0 <= m_done < 4:
                    emit_tail(J, m_done)

            def emit_tail(J, m):
                j = 4 * J + m
                recip = smpool.tile([128, 1], f32, tag="recip", name="recip")
                nc.vector.reciprocal(recip[:], u_ps[J][m][:, D:D + 1])
                y_out = smpool.tile([128, D], f32, tag="yout", name="yout")
                nc.vector.tensor_scalar_mul(y_out[:], u_ps[J][m][:, 0:D],
                                            recip[:, 0:1])
                nc.sync.dma_start(out_d[128 * j:128 * (j + 1), :], y_out[:])

            DEPTH = 5
            pending = []

            def push_pair(J, cp):
                tiles = emit_scores(J, cp)
                pending.append((J, cp, tiles))
                if len(pending) > DEPTH:
                    pJ, pcp, pt = pending.pop(0)
                    emit_U(pJ, pcp, pt)

            pair_idx = 0
            for sb in range(S // 512):
                if sb < TSH // 512:
                    emit_qproj(sb)
                emit_kvload(sb)
                while (pair_idx < len(pairs)
                       and pairs[pair_idx][0] <= sb
                       and 2 * pairs[pair_idx][1] + 1 <= 4 * sb + 3):
                    push_pair(*pairs[pair_idx])
                    pair_idx += 1
            while pair_idx < len(pairs):
                push_pair(*pairs[pair_idx])
                pair_idx += 1
            for pJ, pcp, pt in pending:
                emit_U(pJ, pcp, pt)

    nc.compile()
    return nc


def _get_nc():
    if "nc" not in _CACHE:
        _CACHE["nc"] = _build()
    return _CACHE["nc"]


def _make_masks(p):
    """Two [128,128] tiles: mask_a for even chunk offsets o, mask_b for odd o,
    applied at the diagonal-boundary block (slot m = o//2). Layout [s, t]:
    diag = triu. p=0: (diag, fully-masked); p=1: (all-keep, diag)."""
    triu = np.triu(np.ones((128, 128), np.float32))
    ones = np.ones((128, 128), np.float32)
    zeros = np.zeros((128, 128), np.float32)
    return (triu, zeros) if p == 0 else (ones, triu)


def _make_in_maps(query, key, value, Wq, bq, Wk, bk, Wv, bv):
    import ml_dtypes
    f32 = np.float32
    bf16 = ml_dtypes.bfloat16
    Wq = np.asarray(Wq, f32)
    Wk = np.asarray(Wk, f32)
    Wv = np.asarray(Wv, f32)
    bq = np.asarray(bq, f32)
    bv = np.asarray(bv, f32)
    M = (Wq.T @ Wk).astype(f32)                             # [di, d]
    u = (Wk.T @ bq).astype(f32)                             # [D]
    wva = np.concatenate([Wv.T.astype(f32), np.zeros((D, 2), f32)], axis=1)
    bvrow = np.zeros((128, 258), f32)
    bvrow[0, 0:D] = bv
    bvrow[0, D] = 1.0
    wgt = np.concatenate(
        [M[0:128], M[128:256], wva[0:128], wva[128:256], bvrow],
        axis=1).astype(bf16)
    assert wgt.shape == (128, CW)
    in_maps = []
    for c in range(NCORES):
        n, p = c // 2, c % 2
        mska, mskb = _make_masks(p)
        msk = np.concatenate([mska, mskb], axis=1).astype(bf16)
        blocks = np.arange(NBLK) * 2 + p
        rows = (blocks[:, None] * 128 + np.arange(128)[None, :]).ravel()
        g = np.exp((np.asarray(key[n], f32) @ u) * SCALE)   # [S]
        vgT = (np.asarray(value[n], f32) * g[:, None]).T    # [D, S]
        in_maps.append({
            "qT": np.ascontiguousarray(query[n][rows].T.astype(bf16)),
            "kT": np.ascontiguousarray(key[n].T.astype(bf16)),
            "vg": np.ascontiguousarray(vgT.astype(bf16)),
            "g": np.ascontiguousarray(g.reshape(1, S).astype(bf16)),
            "wgt": np.ascontiguousarray(wgt),
            "msk": np.ascontiguousarray(msk),
        })
    return in_maps


def _gather(results):
    out_full = np.zeros((N, T, D), np.float32)
    for c in range(NCORES):
        n, p = c // 2, c % 2
        shard = results[c]["out"]
        for j in range(NBLK):
            g = 2 * j + p
            out_full[n, 128 * g:128 * (g + 1)] = shard[128 * j:128 * (j + 1)]
    return out_full


def _run(in_maps, trace=False):
    from concourse.bass_utils import run_bass_kernel_spmd
    nc = _get_nc()
    res = run_bass_kernel_spmd(nc, in_maps, core_ids=list(range(NCORES)),
                               trace=trace)
    return res


def kernel(query, key, value, attn_mask=None, Wq=None, bq=None, Wk=None,
           bk=None, Wv=None, bv=None):
    query = np.asarray(query)
    key = np.asarray(key)
    value = np.asarray(value)
    in_maps = _make_in_maps(query, key, value, np.asarray(Wq), np.asarray(bq),
                            np.asarray(Wk), np.asarray(bk), np.asarray(Wv),
                            np.asarray(bv))
    res = _run(in_maps, trace=False)
    return _gather(res.results)


def kernel_profiled(query, key, value, attn_mask=None, Wq=None, bq=None,
                    Wk=None, bk=None, Wv=None, bv=None):
    """Like kernel() but with NTFF tracing; returns (out, BassKernelResults)."""
    in_maps = _make_in_maps(np.asarray(query), np.asarray(key),
                            np.asarray(value), np.asarray(Wq), np.asarray(bq),
                            np.asarray(Wk), np.asarray(bk), np.asarray(Wv),
                            np.asarray(bv))
    res = _run(in_maps, trace=True)
    return _gather(res.results), res


# revision 33
# speedup vs baseline: 1.7949x; 1.7949x over previous
"""Trainium2 Bass kernel for causal attention layer (N=4, T=S=4096, D=256, f32).

Sharding: 8 cores = 4 batches x 2-way split of T. Each batch's 32 query
row-blocks (128 rows each) are split by parity: core parity 0 gets even
global blocks, parity 1 odd blocks. Causal boundaries are enforced by two
per-core [128,128] mask tiles supplied as input data, so the instruction
stream is identical on all 8 cores (no collectives).

Algebraic restructuring vs the naive layer (validated to 9e-7 in f32):
  scores[t,s] = (Wq query_t + bq) . (Wk key_s + bk)
              = query_t . M key_s + beta_s + c_t,   M = Wq^T Wk
  c_t is constant over s -> softmax-invariant -> dropped.
  beta_s = (Wk^T bq) . key_s (+ bq.bk, also invariant): folded on the HOST
  into a per-row scale of value: vg_s = exp(beta_s/16) * [value_s | 1], so
  the device uses RAW key (no k-projection) and RAW scaled value (no
  v-projection); softmax denominator comes from the g column of vg.
  y = (attn @ vg[:, :256]) / denom @ Wv^T + bv   (Wv applied at the END to
  the [t,256] normalized output - 2x fewer projection FLOPs than projecting
  k and v, and only q-side work scales with T).

Device algorithm per core (bf16 matmul operands, f32 PSUM accumulation):
  qMT = M^T @ queryT            [d, t]  (PE + DVE copy)
  kT, vg loaded raw by DMA      [d, s], [s, 258]
  per 512-wide t-superblock J, per 128-row s-chunk:
    scoresT[s, t] = kT_chunk.T @ qMT_block          (PE, 2 d-chunk matmuls)
    attnT = exp(scoresT / 16)                       (ScalarE)
    diagonal-boundary block multiplied by a mask tile (DVE)
    U[m] += attnT[:, block m].T @ vg[chunk]         (PE) -> [t=128, 258]
  Tail per slot (3-stage, interleaved with the pair stream so the PE never
  head-of-line blocks on DVE): A: recip+normalize (DVE); B: 2 PE transposes
  + DVE copy; C: 2 Wv matmuls (PE) + bias add (DVE) + DMA out.
"""
import os
import numpy as np

N, T, S, D = 4, 4096, 4096, 256
NCORES = 8
TSH = T // 2          # 2048 query rows per core
NBLK = TSH // 128     # 16 slots per core
NSB = 4               # superblocks of 512 t rows
SCALE = 1.0 / 16.0    # 1/sqrt(D)

# packed constants layout
# wgt (bf16): [M0|M1|WvT0|WvT1|I]  -> 4*256 + 128 cols
# msk (bf16): [mska|mskb]          -> 256 cols
# bvt (f32):  bv broadcast         -> 256 cols
CW = 4 * 256 + 128

_CACHE = {}


def _build(loop_r=None):
    from contextlib import ExitStack
    import concourse.bass as bass
    import concourse.tile as tile
    from concourse import bacc, mybir

    f32 = mybir.dt.float32
    bf16 = mybir.dt.bfloat16
    fp8 = mybir.dt.float8e4
    DR = mybir.MatmulPerfMode.DoubleRow
    nc = bacc.Bacc("TRN2", target_bir_lowering=False, debug=False,
                   num_devices=NCORES)

    qT_d = nc.dram_tensor("qT", [D, TSH], bf16, kind="ExternalInput")
    kT_d = nc.dram_tensor("kT", [D, S], bf16, kind="ExternalInput")
    vg_d = nc.dram_tensor("vg", [S, 258], bf16, kind="ExternalInput")
    wg_d = nc.dram_tensor("wgt", [128, CW], bf16, kind="ExternalInput")
    bv_d = nc.dram_tensor("bvt", [128, D], f32, kind="ExternalInput")
    mk_d = nc.dram_tensor("msk", [128, 256], bf16, kind="ExternalInput")
    out_d = nc.dram_tensor("out", [TSH, D], f32, kind="ExternalOutput")

    Exp = mybir.ActivationFunctionType.Exp

    with tile.TileContext(nc) as tc, ExitStack() as _loopctx:
        if loop_r is not None:
            _loopctx.enter_context(tc.For_i(0, loop_r, 1))
        with (
            tc.tile_pool(name="const", bufs=1) as cpool,          # persistent
            tc.tile_pool(name="stage", bufs=3) as spool,          # input staging
            tc.tile_pool(name="work", bufs=8) as wpool,           # attn tiles
            tc.tile_pool(name="small", bufs=4) as smpool,         # tail sbuf
            tc.tile_pool(name="ps2", bufs=4, space="PSUM") as pspool,
            tc.tile_pool(name="psu", bufs=1, space="PSUM") as upool,
        ):
            # ---- persistent SBUF tensors -------------------------------
            wgt = cpool.tile([128, CW], bf16, tag="wgt", name="wgt")
            bvt = cpool.tile([128, D], f32, tag="bvt", name="bvt")
            msk = cpool.tile([128, 256], bf16, tag="msk", name="msk")
            qMT_sb = [cpool.tile([128, TSH], bf16, tag=f"qMp{i}", name=f"qMp{i}") for i in range(2)]
            kT_sb = [cpool.tile([128, S], bf16, tag=f"kTp{i}", name=f"kTp{i}") for i in range(2)]
            vq_sb = [cpool.tile([128, 4 * (D + 2)], bf16, tag=f"v{i}", name=f"v{i}")
                     for i in range(S // 512)]
            v_sb = [vq_sb[c // 4][:, (D + 2) * (c % 4):(D + 2) * (c % 4 + 1)]
                    for c in range(S // 128)]

            nc.sync.dma_start(wgt[:, 0:512], wg_d[:, 0:512])
            nc.scalar.dma_start(bvt[:], bv_d[:, :])
            nc.sync.dma_start(wgt[:, 512:CW], wg_d[:, 512:CW])
            nc.scalar.dma_start(msk[:], mk_d[:, :])

            # tiny exp at t=0: hoists the ACT table load under the DMA wait
            warm = cpool.tile([128, 1], f32, tag="warm", name="warm")
            nc.scalar.activation(warm[:], bvt[:, 0:1], Exp, scale=0.0)

            m_sb = [wgt[:, 256 * i:256 * (i + 1)] for i in range(2)]
            wvt_sb = [wgt[:, 512 + 256 * i:512 + 256 * (i + 1)] for i in range(2)]
            ident = wgt[:, 1024:1152]
            mska = msk[:, 0:128]
            mskb = msk[:, 128:256]

            # ---- q projection: qMT[d, t] = M^T @ queryT ----------------
            def emit_qproj(tb):
                qin = [spool.tile([128, 512], bf16, tag=f"qin{i}", name=f"qin{i}") for i in range(2)]
                for i in range(2):
                    nc.sync.dma_start(qin[i][:],
                                      qT_d[128 * i:128 * (i + 1), 512 * tb:512 * (tb + 1)])
                for o in range(2):
                    ps = pspool.tile([128, 512], f32, tag="ps2", name="ps2")
                    for i in range(2):
                        nc.tensor.matmul(ps[:], m_sb[i][:, 128 * o:128 * (o + 1)],
                                         qin[i][:], start=(i == 0), stop=(i == 1))
                    nc.vector.tensor_copy(qMT_sb[o][:, 512 * tb:512 * (tb + 1)],
                                          ps[:])

            # ---- k/v loads: raw, no projection -------------------------
            def emit_kvload(sb):
                for i in range(2):
                    nc.gpsimd.dma_start(
                        kT_sb[i][:, 512 * sb:512 * (sb + 1)],
                        kT_d[128 * i:128 * (i + 1), 512 * sb:512 * (sb + 1)])
                nc.gpsimd.dma_start(
                    vq_sb[sb][:].rearrange("p (b j) -> p b j", b=4),
                    vg_d[512 * sb:512 * (sb + 1), :]
                    .rearrange("(b p) j -> p b j", p=128))

            pairs = [(J, cp) for J in range(NSB) for cp in range(4 * J + 4)]
            u_ps = {}

            def emit_scores(J, cp):
                c0 = 2 * cp
                at = wpool.tile([128, 1024], bf16, tag="att", name="att")
                dga = dgb = None
                for e in range(2):
                    c = c0 + e
                    o = c - 8 * J
                    off = 0 if o < 0 else 128 * (o // 2)
                    sc = pspool.tile([128, 512], f32, tag="ps2", name="ps2")
                    for i in range(2):
                        nc.tensor.matmul(
                            sc[:, off:512],
                            kT_sb[i][:, 128 * c:128 * (c + 1)],
                            qMT_sb[i][:, 512 * J + off:512 * (J + 1)],
                            start=(i == 0), stop=(i == 1))
                    nc.scalar.activation(at[:, 512 * e + off:512 * (e + 1)],
                                         sc[:, off:512], Exp, scale=SCALE)
                    if o >= 0:
                        kk = o // 2
                        dg = wpool.tile([128, 128], bf16,
                                        tag=("dga" if e == 0 else "dgb"),
                                        name=("dga" if e == 0 else "dgb"))
                        nc.vector.tensor_mul(
                            dg[:], at[:, 512 * e + 128 * kk:512 * e + 128 * (kk + 1)],
                            mska[:] if e == 0 else mskb[:])
                        if e == 0:
                            dga = dg
                        else:
                            dgb = dg
                return at, dga, dgb

            tails = []            # (J, m, stage, state)

            def emit_U(J, cp, tiles):
                at, dga, dgb = tiles
                if cp == 0:
                    u_ps[J] = [upool.tile([128, D + 2], f32, tag=f"u{m}", name=f"u{m}")
                               for m in range(4)]
                for e in range(2):
                    c = 2 * cp + e
                    o = c - 8 * J
                    m_min = 0 if o < 0 else o // 2
                    ms = [m for m in range(m_min, 4) if c <= 8 * J + 2 * m + 1]
                    if o >= 0 and ms and ms[0] == o // 2:
                        ms = ms[1:] + ms[:1]      # diag (mask-gated) block last
                    for m in ms:
                        lastc = 8 * J + 2 * m + 1
                        if o >= 0 and m == o // 2:
                            lhsT = (dga if e == 0 else dgb)[:]
                        else:
                            lhsT = at[:, 512 * e + 128 * m:512 * e + 128 * (m + 1)]
                        nc.tensor.matmul(u_ps[J][m][:], lhsT, v_sb[c][:],
                                         start=(c == 0), stop=(c == lastc),
                                         skip_group_check=True)
                m_done = cp - 4 * J       # slot whose accumulation just closed
                if 0 <= m_done < 4:
                    tails.append([J, m_done, 0, None])

            # ---- 3-stage tail pipeline ---------------------------------
            def tail_stage(t):
                J, m, stage, st = t
                j = 4 * J + m
                if stage == 0:
                    recip = smpool.tile([128, 1], f32, tag="recip", name="recip")
                    nc.vector.reciprocal(recip[:], u_ps[J][m][:, D:D + 1])
                    y_bf = smpool.tile([128, D], bf16, tag="ybf", name="ybf")
                    nc.vector.tensor_scalar_mul(y_bf[:], u_ps[J][m][:, 0:D],
                                                recip[:, 0:1])
                    t[3] = y_bf
                elif stage == 1:
                    y_bf = st
                    psw = pspool.tile([128, 512], f32, tag="ps2", name="ps2")
                    psT = psw[:, 0:128].bitcast(bf16)
                    for h in range(2):
                        nc.tensor.transpose(psT[:, 128 * h:128 * (h + 1)],
                                            y_bf[:, 128 * h:128 * (h + 1)],
                                            ident[:])
                    ynT = smpool.tile([128, D], bf16, tag="ynT", name="ynT")
                    nc.vector.tensor_copy(ynT[:], psT[:])
                    t[3] = ynT
                else:
                    ynT = st
                    y_ps = pspool.tile([128, 512], f32, tag="ps2", name="ps2")
                    y_ps = y_ps[:, 0:D]
                    for h in range(2):
                        nc.tensor.matmul(y_ps[:], ynT[:, 128 * h:128 * (h + 1)],
                                         wvt_sb[h][:], start=(h == 0),
                                         stop=(h == 1))
                    y_out = smpool.tile([128, D], f32, tag="yout", name="yout")
                    nc.vector.tensor_add(y_out[:], y_ps[:], bvt[:])
                    nc.sync.dma_start(out_d[128 * j:128 * (j + 1), :], y_out[:])
                t[2] += 1

            def advance_tails(ration=True):
                for t in tails:
                    if t[2] == 0:
                        tail_stage(t)      # stage 0 always runs at once
                for t in tails:
                    if t[2] < 3:
                        tail_stage(t)
                        if ration:
                            break
                while tails and tails[0][2] >= 3:
                    tails.pop(0)

            DEPTH = 5
            pending = []

            def push_pair(J, cp):
                tiles = emit_scores(J, cp)
                pending.append((J, cp, tiles))
                if len(pending) > DEPTH:
                    pJ, pcp, pt = pending.pop(0)
                    emit_U(pJ, pcp, pt)
                advance_tails()

            pair_idx = 0
            for sb in range(S // 512):
                if sb < TSH // 512:
                    emit_qproj(sb)
                emit_kvload(sb)
                while (pair_idx < len(pairs)
                       and pairs[pair_idx][0] <= sb
                       and 2 * pairs[pair_idx][1] + 1 <= 4 * sb + 3):
                    push_pair(*pairs[pair_idx])
                    pair_idx += 1
            while pair_idx < len(pairs):
                push_pair(*pairs[pair_idx])
                pair_idx += 1
            for pJ, pcp, pt in pending:
                emit_U(pJ, pcp, pt)
                advance_tails()
            for _ in range(32):
                advance_tails(ration=False)

    nc.compile()
    return nc


def _get_nc():
    if "nc" not in _CACHE:
        _CACHE["nc"] = _build()
    return _CACHE["nc"]


def _make_masks(p):
    """Two [128,128] tiles: mask_a for even chunk offsets o, mask_b for odd o,
    applied at the diagonal-boundary block (slot m = o//2). Layout [s, t]:
    diag = triu. p=0: (diag, fully-masked); p=1: (all-keep, diag)."""
    triu = np.triu(np.ones((128, 128), np.float32))
    ones = np.ones((128, 128), np.float32)
    zeros = np.zeros((128, 128), np.float32)
    return (triu, zeros) if p == 0 else (ones, triu)


def _make_in_maps(query, key, value, Wq, bq, Wk, bk, Wv, bv):
    import ml_dtypes
    f32 = np.float32
    bf16 = ml_dtypes.bfloat16
    Wq = np.asarray(Wq, f32)
    Wk = np.asarray(Wk, f32)
    Wv = np.asarray(Wv, f32)
    bq = np.asarray(bq, f32)
    bv = np.asarray(bv, f32)
    M = (Wq.T @ Wk).astype(f32)                             # [di, d]
    u = (Wk.T @ bq).astype(f32)                             # [D]
    wvt = Wv.T.astype(f32)                                  # [fi, fo]
    ident = np.eye(128, dtype=f32)
    wgt = np.concatenate(
        [M[0:128], M[128:256], wvt[0:128], wvt[128:256], ident],
        axis=1).astype(bf16)
    assert wgt.shape == (128, CW)
    bvt = np.broadcast_to(bv, (128, D)).astype(f32)
    in_maps = []
    for c in range(NCORES):
        n, p = c // 2, c % 2
        mska, mskb = _make_masks(p)
        msk = np.concatenate([mska, mskb], axis=1).astype(bf16)
        blocks = np.arange(NBLK) * 2 + p
        rows = (blocks[:, None] * 128 + np.arange(128)[None, :]).ravel()
        g = np.exp((np.asarray(key[n], f32) @ u) * SCALE)   # [S]
        vg = np.concatenate(
            [np.asarray(value[n], f32) * g[:, None], g[:, None],
             np.zeros((S, 1), f32)], axis=1).astype(bf16)
        in_maps.append({
            "qT": np.ascontiguousarray(query[n][rows].T.astype(bf16)),
            "kT": np.ascontiguousarray(key[n].T.astype(bf16)),
            "vg": np.ascontiguousarray(vg),
            "wgt": np.ascontiguousarray(wgt),
            "bvt": np.ascontiguousarray(bvt),
            "msk": np.ascontiguousarray(msk),
        })
    return in_maps


def _gather(results):
    out_full = np.zeros((N, T, D), np.float32)
    for c in range(NCORES):
        n, p = c // 2, c % 2
        shard = results[c]["out"]
        for j in range(NBLK):
            g = 2 * j + p
            out_full[n, 128 * g:128 * (g + 1)] = shard[128 * j:128 * (j + 1)]
    return out_full


def _run(in_maps, trace=False):
    from concourse.bass_utils import run_bass_kernel_spmd
    nc = _get_nc()
    res = run_bass_kernel_spmd(nc, in_maps, core_ids=list(range(NCORES)),
                               trace=trace)
    return res


def kernel(query, key, value, attn_mask=None, Wq=None, bq=None, Wk=None,
           bk=None, Wv=None, bv=None):
    query = np.asarray(query)
    key = np.asarray(key)
    value = np.asarray(value)
    in_maps = _make_in_maps(query, key, value, np.asarray(Wq), np.asarray(bq),
                            np.asarray(Wk), np.asarray(bk), np.asarray(Wv),
                            np.asarray(bv))
    res = _run(in_maps, trace=False)
    return _gather(res.results)


def kernel_profiled(query, key, value, attn_mask=None, Wq=None, bq=None,
                    Wk=None, bk=None, Wv=None, bv=None):
    """Like kernel() but with NTFF tracing; returns (out, BassKernelResults)."""
    in_maps = _make_in_maps(np.asarray(query), np.asarray(key),
                            np.asarray(value), np.asarray(Wq), np.asarray(bq),
                            np.asarray(Wk), np.asarray(bk), np.asarray(Wv),
                            np.asarray(bv))
    res = _run(in_maps, trace=True)
    return _gather(res.results), res
